# revision 9
# baseline (speedup 1.0000x reference)
"""Trainium2 Bass kernel for a single GRU-attention decoder step.

Model (H=2048, V=50257, S=64), one token:
    embedded = emb[word]                                   # [1,H]
    aw   = softmax(cat(embedded,h0) @ attn_w.T + attn_b)   # [1,S]
    appl = aw @ encoder_outputs                            # [1,H]
    x    = relu(cat(embedded,appl) @ comb_w.T + comb_b)    # [1,H]
    GRU(x, h0) -> h_new                                    # [1,H]
    out  = log_softmax(h_new @ out_w.T + out_b)            # [1,V]

Distribution over 8 NeuronCores (contraction sharding):
  - core k owns the 256-wide slice I_k of the hidden dim
  - attention stage replicated (tiny weights)
  - comb rows sharded -> x_k, AllGather(x) (2KB)
  - GRU rows sharded -> h_new_k (no collective needed)
  - out_w sharded along H (contraction) -> partial logits, one
    AllReduce of 201KB, then local log-softmax
All GEMVs run on the PE as column-blocked matmuls: stationary [128,128]
weight tiles (host pre-transposed/pre-blocked so every DMA is a natural
2D slice) against [128,1] moving vectors, accumulating in PSUM.
"""

import functools
import os
from contextlib import ExitStack

import numpy as np

H = 2048
V = 50257
S = 64
NCORES = 8
HS = H // NCORES          # 256 hidden slice per core
VP = 50304                # V padded to a multiple of 128
NVC = VP // 128           # 393 column-blocks of logits
NEG_BIG = -1.0e30

LAST_EXEC_NS = None
LAST_RESULTS = None


@functools.lru_cache(maxsize=1)
def _build():
    import concourse.bacc as bacc
    import concourse.mybir as mybir
    import concourse.tile as tile

    f32 = mybir.dt.float32
    AF = mybir.ActivationFunctionType
    AX = mybir.AxisListType

    nc = bacc.Bacc("TRN2", target_bir_lowering=False, debug=False,
                   num_devices=NCORES)

    def din(name, shape):
        return nc.dram_tensor(name, list(shape), f32, kind="ExternalInput").ap()

    def dout(name, shape):
        return nc.dram_tensor(name, list(shape), f32, kind="ExternalOutput").ap()

    cat1 = din("cat1_blk", (128, 32))        # col j = cat(emb,h0)[j*128:(j+1)*128]
    ident = din("ident", (128, 128))
    ones = din("ones_row", (1, 128))
    enc = din("enc", (S, H))
    attnw = din("attnw_blk", (128, 32 * S))  # chunk ic at cols ic*64
    attnb = din("attnb_col", (S, 1))
    combw = din("combw_blk", (128, 32 * HS))  # chunk ic at cols ic*256
    combb = din("combb_blk", (128, 2))
    wih = din("wih_blk", (128, 16 * 3 * HS))  # chunk ic at cols ic*768
    whh = din("whh_blk", (128, 16 * 3 * HS))
    bih = din("bih_blk", (128, 6))
    bhh = din("bhh_blk", (128, 6))
    h0k = din("h0k_blk", (128, 2))
    outw = din("outw", (HS, VP))              # out_w[:, I_k].T, padded cols
    outb = din("outb_blk", (128, NVC))

    o_logits = dout("o_logits", (128, NVC))
    o_hnew = dout("o_hnew", (2, 128))
    o_attnw = dout("o_attnw", (1, S))

    RG = [list(range(NCORES))]

    with tile.TileContext(nc) as tc:
        with ExitStack() as ctx:
            persist = ctx.enter_context(tc.tile_pool(name="persist", bufs=1))
            wsm = ctx.enter_context(tc.tile_pool(name="wsm", bufs=2))
            wout = ctx.enter_context(tc.tile_pool(name="wout", bufs=5))
            work = ctx.enter_context(tc.tile_pool(name="work", bufs=1))
            pss = ctx.enter_context(tc.tile_pool(name="pss", bufs=4, space="PSUM"))
            psl = ctx.enter_context(tc.tile_pool(name="psl", bufs=1, space="PSUM"))
            dram = ctx.enter_context(tc.tile_pool(name="dram", bufs=1, space="DRAM"))

            # ---- persistent small loads (scalar/ACT HWDGE ring) ----
            def pload(ap, shape, name):
                t = persist.tile(list(shape), f32, name=name)
                nc.scalar.dma_start(out=t[:], in_=ap[:])
                return t

            cat1_sb = pload(cat1, (128, 32), "cat1_sb")
            ident_sb = pload(ident, (128, 128), "ident_sb")
            ones_sb = pload(ones, (1, 128), "ones_sb")
            enc_sb = pload(enc, (S, H), "enc_sb")
            attnb_sb = pload(attnb, (S, 1), "attnb_sb")
            combb_sb = pload(combb, (128, 2), "combb_sb")
            bih_sb = pload(bih, (128, 6), "bih_sb")
            bhh_sb = pload(bhh, (128, 6), "bhh_sb")
            h0k_sb = pload(h0k, (128, 2), "h0k_sb")
            outb_sb = pload(outb, (128, NVC), "outb_sb")

            # DRAM bounce buffers for the collectives
            ag_in = dram.tile([2, 128], f32)
            ag_out = dram.tile([16, 128], f32, addr_space="Shared")
            cc_in = dram.tile([128, NVC], f32)
            cc_out = dram.tile([128, NVC], f32, addr_space="Shared")

            # ---------------- attention scores ----------------
            aw_strip = wsm.tile([128, 32 * S], f32, tag="wsm")
            nc.sync.dma_start(out=aw_strip[:], in_=attnw[:])
            sc_ps = pss.tile([S, 1], f32, tag="pss", name="sc_ps")
            for ic in range(32):
                nc.tensor.matmul(sc_ps[:], aw_strip[:, ic * S:(ic + 1) * S],
                                 cat1_sb[:, ic:ic + 1],
                                 start=(ic == 0), stop=(ic == 31))
            sc_sb = work.tile([S, 1], f32)
            nc.vector.tensor_add(sc_sb[:], sc_ps[:], attnb_sb[:])
            scT_ps = pss.tile([1, S], f32, tag="pss", name="scT_ps")
            nc.tensor.transpose(scT_ps[:], sc_sb[:], ident_sb[:S, :S])
            scT_sb = work.tile([1, S], f32)
            nc.vector.tensor_copy(scT_sb[:], scT_ps[:])
            mx = work.tile([1, 1], f32)
            nc.vector.reduce_max(mx[:], scT_sb[:], AX.X)
            nmx = work.tile([1, 1], f32)
            nc.vector.tensor_scalar_mul(nmx[:], mx[:], -1.0)
            e_att = work.tile([1, S], f32)
            se = work.tile([1, 1], f32)
            nc.scalar.activation(e_att[:], scT_sb[:], AF.Exp,
                                 bias=nmx[:], scale=1.0, accum_out=se[:])
            rse = work.tile([1, 1], f32)
            nc.vector.reciprocal(rse[:], se[:])
            aw_row = work.tile([1, S], f32)
            nc.vector.tensor_scalar_mul(aw_row[:], e_att[:], rse[:])
            nc.scalar.dma_start(out=o_attnw[:], in_=aw_row[:])
            awT_ps = pss.tile([S, 1], f32, tag="pss", name="awT_ps")
            nc.tensor.transpose(awT_ps[:], aw_row[:], ident_sb[:1, :1])
            aw_col = work.tile([S, 1], f32)
            nc.vector.tensor_copy(aw_col[:], awT_ps[:])

            # ---------------- attn applied ----------------
            aa_ps = pss.tile([128, 16], f32, tag="pss", name="aa_ps")
            for hc in range(16):
                nc.tensor.matmul(aa_ps[:, hc:hc + 1],
                                 enc_sb[:, hc * 128:(hc + 1) * 128],
                                 aw_col[:], start=True, stop=True)
            aa_sb = work.tile([128, 16], f32)
            nc.vector.tensor_copy(aa_sb[:], aa_ps[:])

            # ---------------- comb -> x_k ----------------
            # NOTE: PSUM accumulation groups within one 2KB zero region must be
            # strictly sequential (start..stop complete before the next start),
            # so the contraction loop is innermost everywhere below.
            cb_ps = pss.tile([128, 2], f32, tag="pss", name="cb_ps")
            cw_strips = []
            for t in range(2):
                cw_strip = wsm.tile([128, 16 * HS], f32, tag="wsm", name="cw_strip")
                nc.sync.dma_start(out=cw_strip[:],
                                  in_=combw[:, t * 16 * HS:(t + 1) * 16 * HS])
                cw_strips.append(cw_strip)
            for oc in range(2):
                for ic in range(32):
                    icl = ic % 16
                    rhs = cat1_sb[:, ic:ic + 1] if ic < 16 else \
                        aa_sb[:, ic - 16:ic - 15]
                    nc.tensor.matmul(cb_ps[:, oc:oc + 1],
                                     cw_strips[ic // 16][:, icl * HS + oc * 128:
                                                         icl * HS + (oc + 1) * 128],
                                     rhs, start=(ic == 0), stop=(ic == 31))
            xk_pre = work.tile([128, 2], f32)
            nc.vector.tensor_add(xk_pre[:], cb_ps[:], combb_sb[:])
            xk_sb = work.tile([128, 2], f32)
            nc.vector.tensor_scalar_max(xk_sb[:], xk_pre[:], 0.0)

            # ---------------- AllGather x ----------------
            xkT_ps = pss.tile([2, 128], f32, tag="pss", name="xkT_ps")
            nc.tensor.transpose(xkT_ps[:], xk_sb[:], ident_sb[:])
            xkT_sb = work.tile([2, 128], f32)
            nc.vector.tensor_copy(xkT_sb[:], xkT_ps[:])
            nc.scalar.dma_start(out=ag_in[:], in_=xkT_sb[:])
            nc.gpsimd.collective_compute(
                "AllGather", mybir.AluOpType.bypass,
                replica_groups=RG, ins=[ag_in.opt()], outs=[ag_out.opt()])
            xT_sb = work.tile([16, 128], f32)
            nc.scalar.dma_start(out=xT_sb[:], in_=ag_out[:])
            x_ps = pss.tile([128, 16], f32, tag="pss", name="x_ps")
            nc.tensor.transpose(x_ps[:], xT_sb[:], ident_sb[:16, :16])
            x_sb = work.tile([128, 16], f32)
            nc.vector.tensor_copy(x_sb[:], x_ps[:])

            # ---------------- GRU gates ----------------
            gi_ps = pss.tile([128, 6], f32, tag="pss", name="gi_ps")
            gh_ps = pss.tile([128, 6], f32, tag="pss", name="gh_ps")
            for (w_ap, ps, rhs_sb, rhs_off) in (
                    (wih, gi_ps, x_sb, 0), (whh, gh_ps, cat1_sb, 16)):
                g_strips = []
                for t in range(2):
                    g_strip = wsm.tile([128, 8 * 3 * HS], f32, tag="wsm",
                                       name="g_strip")
                    nc.sync.dma_start(
                        out=g_strip[:],
                        in_=w_ap[:, t * 8 * 3 * HS:(t + 1) * 8 * 3 * HS])
                    g_strips.append(g_strip)
                for oc in range(6):
                    for ic in range(16):
                        icl = ic % 8
                        rhs = rhs_sb[:, rhs_off + ic:rhs_off + ic + 1]
                        nc.tensor.matmul(
                            ps[:, oc:oc + 1],
                            g_strips[ic // 8][:, icl * 768 + oc * 128:
                                              icl * 768 + (oc + 1) * 128],
                            rhs, start=(ic == 0), stop=(ic == 15))
            gi_sb = work.tile([128, 6], f32)
            nc.vector.tensor_add(gi_sb[:], gi_ps[:], bih_sb[:])
            gh_sb = work.tile([128, 6], f32)
            nc.vector.tensor_add(gh_sb[:], gh_ps[:], bhh_sb[:])

            rpre = work.tile([128, 2], f32)
            nc.vector.tensor_add(rpre[:], gi_sb[:, 0:2], gh_sb[:, 0:2])
            r_sb = work.tile([128, 2], f32)
            nc.scalar.activation(r_sb[:], rpre[:], AF.Sigmoid)
            zpre = work.tile([128, 2], f32)
            nc.vector.tensor_add(zpre[:], gi_sb[:, 2:4], gh_sb[:, 2:4])
            z_sb = work.tile([128, 2], f32)
            nc.scalar.activation(z_sb[:], zpre[:], AF.Sigmoid)
            rn = work.tile([128, 2], f32)
            nc.vector.tensor_mul(rn[:], r_sb[:], gh_sb[:, 4:6])
            npre = work.tile([128, 2], f32)
            nc.vector.tensor_add(npre[:], gi_sb[:, 4:6], rn[:])
            n_sb = work.tile([128, 2], f32)
            nc.scalar.activation(n_sb[:], npre[:], AF.Tanh)
            d_sb = work.tile([128, 2], f32)
            nc.vector.tensor_sub(d_sb[:], h0k_sb[:], n_sb[:])
            zd_sb = work.tile([128, 2], f32)
            nc.vector.tensor_mul(zd_sb[:], z_sb[:], d_sb[:])
            h_sb = work.tile([128, 2], f32)
            nc.vector.tensor_add(h_sb[:], n_sb[:], zd_sb[:])

            hT_ps = pss.tile([2, 128], f32, tag="pss", name="hT_ps")
            nc.tensor.transpose(hT_ps[:], h_sb[:], ident_sb[:])
            hT_sb = work.tile([2, 128], f32)
            nc.vector.tensor_copy(hT_sb[:], hT_ps[:])
            nc.scalar.dma_start(out=o_hnew[:], in_=hT_sb[:])

            # ---------------- output projection (partial logits) ----------------
            LP_ps = psl.tile([128, NVC], f32)
            for s in range(13):
                cols = 4096 if s < 12 else VP - 12 * 4096    # 1152 tail
                nvt = cols // 128
                wts = []
                for i in range(2):
                    wt = wout.tile([128, 4096], f32, tag="wout", name="wt")
                    nc.sync.dma_start(
                        out=wt[:, :cols],
                        in_=outw[i * 128:(i + 1) * 128,
                                 s * 4096:s * 4096 + cols])
                    wts.append(wt)
                for vt in range(nvt):
                    c = s * 32 + vt
                    for i in range(2):
                        nc.tensor.matmul(LP_ps[:, c:c + 1],
                                         wts[i][:, vt * 128:(vt + 1) * 128],
                                         h_sb[:, i:i + 1],
                                         start=(i == 0), stop=(i == 1))
            LPp_sb = work.tile([128, NVC], f32)
            nc.vector.tensor_copy(LPp_sb[:], LP_ps[:])
            nc.scalar.dma_start(out=cc_in[:], in_=LPp_sb[:])
            nc.gpsimd.collective_compute(
                "AllReduce", mybir.AluOpType.add,
                replica_groups=RG, ins=[cc_in.opt()], outs=[cc_out.opt()])
            L_sb = work.tile([128, NVC], f32)
            nc.scalar.dma_start(out=L_sb[:], in_=cc_out[:])

            # ---------------- log_softmax ----------------
            Lb_sb = work.tile([128, NVC], f32)
            nc.vector.tensor_add(Lb_sb[:], L_sb[:], outb_sb[:])
            rmax = work.tile([128, 1], f32)
            nc.vector.reduce_max(rmax[:], Lb_sb[:], AX.X)
            rmT_ps = pss.tile([1, 128], f32, tag="pss", name="rmT_ps")
            nc.tensor.transpose(rmT_ps[:], rmax[:], ident_sb[:])
            rmT_sb = work.tile([1, 128], f32)
            nc.vector.tensor_copy(rmT_sb[:], rmT_ps[:])
            gmax = work.tile([1, 1], f32)
            nc.vector.reduce_max(gmax[:], rmT_sb[:], AX.X)
            ngmax = work.tile([1, 1], f32)
            nc.vector.tensor_scalar_mul(ngmax[:], gmax[:], -1.0)
            nb_ps = pss.tile([128, 1], f32, tag="pss", name="nb_ps")
            nc.tensor.matmul(nb_ps[:], ones_sb[:], ngmax[:], start=True, stop=True)
            nb_sb = work.tile([128, 1], f32)
            nc.vector.tensor_copy(nb_sb[:], nb_ps[:])
            e_sb = work.tile([128, NVC], f32)
            rsum = work.tile([128, 1], f32)
            nc.scalar.activation(e_sb[:], Lb_sb[:], AF.Exp,
                                 bias=nb_sb[:], scale=1.0, accum_out=rsum[:])
            rsT_ps = pss.tile([1, 128], f32, tag="pss", name="rsT_ps")
            nc.tensor.transpose(rsT_ps[:], rsum[:], ident_sb[:])
            rsT_sb = work.tile([1, 128], f32)
            nc.vector.tensor_copy(rsT_sb[:], rsT_ps[:])
            ssum = work.tile([1, 1], f32)
            nc.vector.reduce_sum(ssum[:], rsT_sb[:], AX.X)
            lnS = work.tile([1, 1], f32)
            nc.scalar.activation(lnS[:], ssum[:], AF.Ln)
            nshift = work.tile([1, 1], f32)
            nc.vector.tensor_sub(nshift[:], ngmax[:], lnS[:])
            nsb_ps = pss.tile([128, 1], f32, tag="pss", name="nsb_ps")
            nc.tensor.matmul(nsb_ps[:], ones_sb[:], nshift[:], start=True, stop=True)
            nsb_sb = work.tile([128, 1], f32)
            nc.vector.tensor_copy(nsb_sb[:], nsb_ps[:])
            final_sb = work.tile([128, NVC], f32)
            nc.vector.tensor_scalar_add(final_sb[:], Lb_sb[:], nsb_sb[:])
            nc.scalar.dma_start(out=o_logits[:], in_=final_sb[:])

    nc.compile()
    return nc


def _blk(vec, p=128):
    """[n*p] -> [p, n] column-chunk block layout (col j = vec[j*p:(j+1)*p])."""
    v = np.ascontiguousarray(vec, dtype=np.float32).reshape(-1)
    n = v.size // p
    return np.ascontiguousarray(v.reshape(n, p).T)


def _wblk(wT, p=128):
    """[n*p, m] -> [p, n*m]: chunk j of rows -> cols j*m:(j+1)*m."""
    wT = np.ascontiguousarray(wT, dtype=np.float32)
    n = wT.shape[0] // p
    m = wT.shape[1]
    return np.ascontiguousarray(
        wT.reshape(n, p, m).transpose(1, 0, 2).reshape(p, n * m))


def prepare_in_maps(word, hidden, encoder_outputs, emb, attn_w, attn_b,
                    comb_w, comb_b, w_ih, w_hh, b_ih, b_hh, out_w, out_b):
    f = np.float32
    h0 = np.asarray(hidden, f).reshape(H)
    widx = int(np.asarray(word).reshape(-1)[0])
    embr = np.asarray(emb[widx], f).reshape(H)
    enc = np.ascontiguousarray(np.asarray(encoder_outputs, f))
    attn_w = np.asarray(attn_w, f)
    comb_w = np.asarray(comb_w, f)
    w_ih = np.asarray(w_ih, f)
    w_hh = np.asarray(w_hh, f)
    out_w = np.asarray(out_w, f)
    out_b = np.asarray(out_b, f).reshape(-1)

    cat1 = np.concatenate([embr, h0])
    cat1_blk = _blk(cat1)                          # [128, 32]
    ident = np.eye(128, dtype=f)
    ones_row = np.ones((1, 128), f)
    attnw_blk = _wblk(attn_w.T)                    # [128, 32*64]
    attnb_col = np.asarray(attn_b, f).reshape(S, 1)
    outb_pad = np.full(VP, NEG_BIG, f)
    outb_pad[:V] = out_b
    outb_blk = _blk(outb_pad)                      # [128, 393]

    common = dict(cat1_blk=cat1_blk, ident=ident, ones_row=ones_row,
                  enc=enc, attnw_blk=attnw_blk, attnb_col=attnb_col,
                  outb_blk=outb_blk)

    in_maps = []
    for k in range(NCORES):
        lo, hi = k * HS, (k + 1) * HS
        rows_k = np.r_[lo:hi, H + lo:H + hi, 2 * H + lo:2 * H + hi]
        outw_pad = np.zeros((HS, VP), f)
        outw_pad[:, :V] = out_w[:, lo:hi].T
        m = dict(common)
        m["combw_blk"] = _wblk(comb_w[lo:hi, :].T)       # [128, 32*256]
        m["combb_blk"] = _blk(np.asarray(comb_b, f)[lo:hi])
        m["wih_blk"] = _wblk(w_ih[rows_k, :].T)          # [128, 16*768]
        m["whh_blk"] = _wblk(w_hh[rows_k, :].T)
        m["bih_blk"] = _blk(np.asarray(b_ih, f)[rows_k])
        m["bhh_blk"] = _blk(np.asarray(b_hh, f)[rows_k])
        m["h0k_blk"] = _blk(h0[lo:hi])
        m["outw"] = outw_pad
        in_maps.append(m)
    return in_maps


def assemble_outputs(r):
    logits = np.ascontiguousarray(
        np.asarray(r[0]["o_logits"]).T.reshape(-1)[:V]).reshape(1, V)
    h_new = np.concatenate(
        [np.asarray(r[k]["o_hnew"]).reshape(-1) for k in range(NCORES)]
    ).reshape(1, 1, H)
    attnw = np.asarray(r[0]["o_attnw"]).reshape(1, S)
    return logits, h_new, attnw


def kernel(word, hidden, encoder_outputs, emb, attn_w, attn_b, comb_w,
           comb_b, w_ih, w_hh, b_ih, b_hh, out_w, out_b):
    global LAST_EXEC_NS, LAST_RESULTS
    from concourse.bass_utils import run_bass_kernel_spmd

    in_maps = prepare_in_maps(word, hidden, encoder_outputs, emb, attn_w,
                              attn_b, comb_w, comb_b, w_ih, w_hh, b_ih,
                              b_hh, out_w, out_b)
    nc = _build()
    res = run_bass_kernel_spmd(nc, in_maps, list(range(NCORES)))
    LAST_EXEC_NS = res.exec_time_ns
    LAST_RESULTS = res
    return assemble_outputs(res.results)
